# revision 12
# baseline (speedup 1.0000x reference)
"""Trainium2 Bass kernel for nn_Decoder (4-layer transformer decoder with
pointer-generator head).

Sharding: 8 cores = 4 batches (data parallel) x 2 sequence halves.
Per layer a pair-AllGather shares the residual-stream halves so both cores
can compute full-sequence K/V for self-attention. Cross-attention K/V is
computed from enc_output locally (replicated within the pair, no comms).

Device layout: activations are kept feature-major (x^T: [D, tokens]) so every
matmul contracts over the partition dimension with no on-device transposes.
Attention computes S^T = K Q^T; softmax denominators come from a ones-column
appended to V (row 64 of the AV psum); per-token broadcasts are materialized
with K=1 matmuls. All matmuls run as float32r (fp32 storage, ~FP22 multiply).
"""

import numpy as np

L, D, H, F = 4, 512, 8, 2048
B, SQ, SK = 4, 512, 1024
DH = D // H
P = 128
NCORES = 8
QH = SQ // 2          # 256 tokens per core
EPS = 1e-6
MASK_COEF = -240000.0  # (mask*-240000 + S) then exp(x/8) == exp(S/8 - 30000*mask)
PAIRS = [[0, 1], [2, 3], [4, 5], [6, 7]]

_CACHE = {}


def _build(pm_zero: bool):
    import concourse.bass as bass
    import concourse.mybir as mybir
    import concourse.tile as tile
    from concourse import bacc
    from concourse.masks import make_identity

    f32 = mybir.dt.float32
    f32r = mybir.dt.float32r
    AF = mybir.ActivationFunctionType
    OP = mybir.AluOpType

    def r(ap):
        return ap.bitcast(f32r)

    nc = bacc.Bacc("TRN2", target_bir_lowering=False, debug=False,
                   num_devices=NCORES)

    # ---- DRAM I/O ----
    d_xT = nc.dram_tensor("xT_in", [D, SQ], f32r, kind="ExternalInput")
    d_x0T = nc.dram_tensor("x0T_half", [D, QH], f32r, kind="ExternalInput")
    d_encT = nc.dram_tensor("encT_in", [D, SK], f32r, kind="ExternalInput")
    d_encpg = nc.dram_tensor("enc_pg", [H, 8, P, DH], f32r, kind="ExternalInput")
    d_maskT = nc.dram_tensor("maskT_in", [SQ, QH], f32, kind="ExternalInput")
    d_pm = nc.dram_tensor("pm_bias", [P, 8], f32, kind="ExternalInput")
    d_sm = nc.dram_tensor("smalls", [L, P, 72], f32, kind="ExternalInput")
    d_pgw = nc.dram_tensor("pg_w", [P, 16], f32r, kind="ExternalInput")
    dw = {}
    for nm in ["sa_wq", "sa_wk", "sa_wv", "sa_wo",
               "ca_wq", "ca_wk", "ca_wv", "ca_wo"]:
        dw[nm] = nc.dram_tensor(nm, [L, D, D], f32r, kind="ExternalInput")
    dw["ffn_w1"] = nc.dram_tensor("ffn_w1", [L, D, F], f32r, kind="ExternalInput")
    dw["ffn_w2"] = nc.dram_tensor("ffn_w2", [L, F, D], f32r, kind="ExternalInput")

    d_out = nc.dram_tensor("out_half", [QH, D], f32, kind="ExternalOutput")
    d_p = nc.dram_tensor("p_half", [QH, 1], f32, kind="ExternalOutput")

    cc_in = [nc.dram_tensor(f"cc_in{l}", [D, QH], f32r) for l in range(L - 1)]
    cc_out = [nc.dram_tensor(f"cc_out{l}", [2 * D, QH], f32r) for l in range(L - 1)]

    from contextlib import ExitStack
    with tile.TileContext(nc) as tc, ExitStack() as ctx, \
            nc.allow_low_precision(reason="float32r tiles hold fp32 bits"):
        pw = ctx.enter_context(tc.tile_pool(name="w", bufs=4))
        pa = ctx.enter_context(tc.tile_pool(name="act", bufs=2))
        pp = ctx.enter_context(tc.tile_pool(name="pers", bufs=1))
        psA = ctx.enter_context(tc.tile_pool(name="psA", bufs=2, space="PSUM"))
        psB = ctx.enter_context(tc.tile_pool(name="psB", bufs=4, space="PSUM"))

        # ---- persistent tiles ----
        t_mask = pp.tile([P, 4, QH], f32)
        nc.sync.dma_start(t_mask[:], d_maskT.ap().rearrange("(k p) q -> p k q", p=P))
        t_encT = pp.tile([P, 4, SK], f32r)
        nc.sync.dma_start(t_encT[:], d_encT.ap().rearrange("(k p) t -> p k t", p=P))
        t_scr = pp.tile([P, P], f32)
        t_ones = pp.tile([P, P], f32r)
        nc.gpsimd.memset(t_scr[:], 1.0)
        nc.vector.tensor_copy(t_ones[:], t_scr[:])
        t_scr2 = pp.tile([P, P], f32)
        t_ident = pp.tile([P, P], f32r)
        make_identity(nc, t_scr2[:])
        nc.vector.tensor_copy(t_ident[:], t_scr2[:])
        t_pm = pp.tile([P, 8], f32)
        nc.sync.dma_start(t_pm[:], d_pm.ap())
        t_pgw = pp.tile([P, 16], f32r)
        nc.sync.dma_start(t_pgw[:], d_pgw.ap())
        t_ctxT = pp.tile([P, 4, QH], f32r)

        # initial residual = own half of x^T
        resid = pa.tile([P, 4, QH], f32r, tag="resid")
        nc.sync.dma_start(resid[:], d_x0T.ap().rearrange("(k p) q -> p k q", p=P))

        def load_w(dram_ap):
            t = pw.tile([P, 4, 512], f32r, tag="w")
            nc.sync.dma_start(t[:], dram_ap.rearrange("(k p) n -> p k n", p=P))
            return t

        def layernorm(t_y, t_sm, gcol, bcol):
            """t_y: [P,4,QH] pre-norm; returns new resid tile."""
            t_y2 = pa.tile([P, 4, QH], f32r, tag="y2", bufs=1)
            nc.scalar.square(t_y2[:].rearrange("p a b -> p (a b)"),
                             t_y[:].rearrange("p a b -> p (a b)"))
            row = psB.tile([P, 512], f32, tag="mm")
            for ko in range(4):
                nc.tensor.matmul(row[0:1, 0:QH], r(t_ones[:, 0:1]),
                                 r(t_y[:, ko, :]), start=(ko == 0), stop=(ko == 3))
            for ko in range(4):
                nc.tensor.matmul(row[0:1, QH:2 * QH], r(t_ones[:, 0:1]),
                                 r(t_y2[:, ko, :]), start=(ko == 0), stop=(ko == 3))
            t_r = pa.tile([1, 2, QH], f32r, tag="lnr")
            # m = ssum/D
            nc.vector.tensor_scalar_mul(t_r[0:1, 0, :], row[0:1, 0:QH], 1.0 / D)
            # var = ssq/D - m^2
            nc.vector.tensor_tensor(out=t_r[0:1, 1, :], in0=t_r[0:1, 0, :],
                                    in1=t_r[0:1, 0, :], op=OP.mult)
            nc.vector.scalar_tensor_tensor(out=t_r[0:1, 1, :],
                                           in0=row[0:1, QH:2 * QH],
                                           scalar=1.0 / D, in1=t_r[0:1, 1, :],
                                           op0=OP.mult, op1=OP.subtract)
            # rs = 1/sqrt(var+eps)
            nc.vector.tensor_scalar_add(t_r[0:1, 1, :], t_r[0:1, 1, :], EPS)
            nc.scalar.activation(t_r[0:1, 1, :], t_r[0:1, 1, :], AF.Sqrt)
            nc.vector.reciprocal(t_r[0:1, 1, :], t_r[0:1, 1, :])
            # broadcast [m | rs] -> [128, 2*QH]
            bc = psB.tile([P, 512], f32, tag="mm")
            nc.tensor.matmul(bc[:, 0:2 * QH], r(t_ones[0:1, 0:P]),
                             r(t_r[0:1, :, :].rearrange("p a b -> p (a b)")),
                             start=True, stop=True)
            newr = pa.tile([P, 4, QH], f32r, tag="resid")
            t1 = pa.tile([P, 4, QH], f32, tag="lnt", bufs=1)
            for mo in range(4):
                nc.vector.tensor_tensor(out=t1[:, mo, :], in0=t_y[:, mo, :],
                                        in1=bc[:, 0:QH], op=OP.subtract)
                nc.vector.tensor_tensor(out=t1[:, mo, :], in0=t1[:, mo, :],
                                        in1=bc[:, QH:2 * QH], op=OP.mult)
                nc.vector.tensor_scalar(out=newr[:, mo, :], in0=t1[:, mo, :],
                                        scalar1=t_sm[:, gcol + mo:gcol + mo + 1],
                                        scalar2=t_sm[:, bcol + mo:bcol + mo + 1],
                                        op0=OP.mult, op1=OP.add)
            return newr

        def attention(l, wq_d, wk_d, wv_d, wo_d, t_sm, bq_c, bk_c, co_c,
                      kv_src, nt, is_sa, save_ctx, resid):
            """one MHA block; returns t_y (pre-LN = attn_out + c_o + resid)."""
            Tk = nt * P
            t_wq = load_w(wq_d)
            t_wk = load_w(wk_d)
            t_wv = load_w(wv_d)
            # qT [P, 4, QH]
            t_qT = pa.tile([P, 4, QH], f32r, tag="qT")
            for mo in range(4):
                ps = psB.tile([P, 512], f32, tag="mm")
                for ko in range(4):
                    nc.tensor.matmul(ps[:, 0:QH],
                                     r(t_wq[:, ko, mo * P:(mo + 1) * P]),
                                     r(resid[:, ko, :]),
                                     start=(ko == 0), stop=(ko == 3))
                nc.scalar.activation(t_qT[:, mo, :], ps[:, 0:QH], AF.Identity,
                                     bias=t_sm[:, bq_c + mo:bq_c + mo + 1])
            # kT [P, 4, Tk]
            t_kT = pa.tile([P, 4, SK], f32r, tag="kT", bufs=1)
            for mo in range(4):
                for hh in range(Tk // 512):
                    ps = psB.tile([P, 512], f32, tag="mm")
                    for ko in range(4):
                        nc.tensor.matmul(ps[:, :],
                                         r(t_wk[:, ko, mo * P:(mo + 1) * P]),
                                         r(kv_src[:, ko, hh * 512:(hh + 1) * 512]),
                                         start=(ko == 0), stop=(ko == 3))
                    nc.scalar.activation(t_kT[:, mo, hh * 512:(hh + 1) * 512],
                                         ps[:, :], AF.Identity,
                                         bias=t_sm[:, bk_c + mo:bk_c + mo + 1])
            # v' [P, nt, H, 65] token-major with ones column
            t_vp = pa.tile([P, 8, H, DH + 1], f32r, tag="vp", bufs=1)
            nc.vector.tensor_scalar(
                out=t_vp[:, 0:nt, :, DH:DH + 1],
                in0=t_ones[:, 0:nt * H].rearrange("p (a b c) -> p a b c",
                                                  b=H, c=1),
                scalar1=0.0, scalar2=1.0, op0=OP.mult, op1=OP.add)
            for mo in range(nt):
                ps = psB.tile([P, 512], f32, tag="mm")
                for ko in range(4):
                    nc.tensor.matmul(ps[:, :],
                                     r(kv_src[:, ko, mo * P:(mo + 1) * P]),
                                     r(t_wv[:, ko, :]),
                                     start=(ko == 0), stop=(ko == 3))
                nc.vector.tensor_copy(t_vp[:, mo, :, 0:DH],
                                      ps[:, :].rearrange("p (h d) -> p h d", d=DH))
            t_wo = load_w(wo_d)
            # per-head scores/softmax/AV
            t_O = pa.tile([P, 4, QH], f32r, tag="Ofull")
            t_rd = pa.tile([P, H, QH], f32r, tag="rd", bufs=1)
            for h in range(H):
                base = (h % 2) * DH
                moh = h // 2
                es = pa.tile([P, 8, QH], f32r, tag="es")
                for g in range(nt // 4):
                    sp = psA.tile([P, 1024], f32, tag="sc")
                    for kc in range(4):
                        kcg = g * 4 + kc
                        nc.tensor.matmul(
                            sp[:, kc * QH:(kc + 1) * QH],
                            r(t_kT[base:base + DH, moh, kcg * P:(kcg + 1) * P]),
                            r(t_qT[base:base + DH, moh, :]),
                            start=True, stop=True)
                    spv = sp[:, :].rearrange("p (k q) -> p k q", q=QH)
                    if is_sa:
                        nc.vector.scalar_tensor_tensor(
                            out=es[:, g * 4:(g + 1) * 4, :], in0=t_mask[:, :, :],
                            scalar=MASK_COEF, in1=spv,
                            op0=OP.mult, op1=OP.add)
                        nc.scalar.activation(es[:, g * 4:(g + 1) * 4, :],
                                             es[:, g * 4:(g + 1) * 4, :],
                                             AF.Exp, scale=0.125)
                    elif pm_zero:
                        nc.scalar.activation(es[:, g * 4:(g + 1) * 4, :], spv,
                                             AF.Exp, scale=0.125)
                    else:
                        for kc in range(4):
                            kcg = g * 4 + kc
                            nc.scalar.activation(
                                es[:, kcg, :], sp[:, kc * QH:(kc + 1) * QH],
                                AF.Exp, scale=0.125,
                                bias=t_pm[:, kcg:kcg + 1])
                op = psB.tile([P, 512], f32, tag="mm")
                for kc in range(nt):
                    nc.tensor.matmul(op[0:DH + 1, 0:QH], r(t_vp[:, kc, h, :]),
                                     r(es[:, kc, :]),
                                     start=(kc == 0), stop=(kc == nt - 1))
                nc.vector.reciprocal(t_rd[64:65, h, :], op[64:65, 0:QH])
                bc = psB.tile([P, 512], f32, tag="mm")
                nc.tensor.matmul(bc[0:DH, 0:QH], r(t_ones[64:65, 0:DH]),
                                 r(t_rd[64:65, h, :]), start=True, stop=True)
                rb = pa.tile([DH, QH], f32, tag="rb")
                nc.vector.tensor_copy(rb[:], bc[0:DH, 0:QH])
                nc.vector.tensor_tensor(out=t_O[base:base + DH, moh, :],
                                        in0=op[0:DH, 0:QH], in1=rb[:],
                                        op=OP.mult)
                if save_ctx:
                    t_ep = pa.tile([P, 8, DH], f32r, tag="ep")
                    nc.sync.dma_start(t_ep[:],
                                      d_encpg[h].rearrange("k p d -> p k d"))
                    cp = psB.tile([P, 512], f32, tag="mm")
                    for kc in range(8):
                        nc.tensor.matmul(cp[0:DH, 0:QH], r(t_ep[:, kc, :]),
                                         r(es[:, kc, :]),
                                         start=(kc == 0), stop=(kc == 7))
                    nc.vector.tensor_tensor(out=t_ctxT[base:base + DH, moh, :],
                                            in0=cp[0:DH, 0:QH],
                                            in1=rb[:], op=OP.mult)
            # wo + folded bias + residual
            t_y = pa.tile([P, 4, QH], f32r, tag="y")
            for mo in range(4):
                ps = psB.tile([P, 512], f32, tag="mm")
                for ko in range(4):
                    nc.tensor.matmul(ps[:, 0:QH],
                                     r(t_wo[:, ko, mo * P:(mo + 1) * P]),
                                     r(t_O[:, ko, :]),
                                     start=(ko == 0), stop=(ko == 3))
                nc.vector.scalar_tensor_tensor(
                    out=t_y[:, mo, :], in0=ps[:, 0:QH],
                    scalar=t_sm[:, co_c + mo:co_c + mo + 1],
                    in1=resid[:, mo, :], op0=OP.add, op1=OP.add)
            return t_y

        # ================= layer loop =================
        for l in range(L):
            t_xT = pa.tile([P, 4, SQ], f32r, tag="xTfull")
            if l == 0:
                nc.sync.dma_start(t_xT[:],
                                  d_xT.ap().rearrange("(k p) t -> p k t", p=P))
            else:
                nc.sync.dma_start(
                    t_xT[:, :, 0:QH],
                    cc_out[l - 1][0:D, :].rearrange("(k p) q -> p k q", p=P))
                nc.sync.dma_start(
                    t_xT[:, :, QH:SQ],
                    cc_out[l - 1][D:2 * D, :].rearrange("(k p) q -> p k q", p=P))
            t_sm = pa.tile([P, 72], f32, tag="sm")
            nc.sync.dma_start(t_sm[:], d_sm[l])

            # self attention
            t_y = attention(l, dw["sa_wq"][l], dw["sa_wk"][l], dw["sa_wv"][l],
                            dw["sa_wo"][l], t_sm, 0, 4, 8,
                            t_xT, 4, True, False, resid)
            resid = layernorm(t_y, t_sm, 44, 48)
            # cross attention
            t_y = attention(l, dw["ca_wq"][l], dw["ca_wk"][l], dw["ca_wv"][l],
                            dw["ca_wo"][l], t_sm, 12, 16, 20,
                            t_encT, 8, False, (l == L - 1), resid)
            resid = layernorm(t_y, t_sm, 52, 56)
            # ffn
            t_h = pa.tile([P, 16, QH], f32r, tag="hT", bufs=1)
            for p4 in range(4):
                t_w1 = pw.tile([P, 4, 512], f32r, tag="w")
                nc.sync.dma_start(
                    t_w1[:],
                    dw["ffn_w1"][l, :, p4 * 512:(p4 + 1) * 512]
                    .rearrange("(k p) n -> p k n", p=P))
                for mi in range(4):
                    mo = p4 * 4 + mi
                    ps = psB.tile([P, 512], f32, tag="mm")
                    for ko in range(4):
                        nc.tensor.matmul(ps[:, 0:QH],
                                         r(t_w1[:, ko, mi * P:(mi + 1) * P]),
                                         r(resid[:, ko, :]),
                                         start=(ko == 0), stop=(ko == 3))
                    nc.scalar.activation(t_h[:, mo, :], ps[:, 0:QH], AF.Relu,
                                         bias=t_sm[:, 24 + mo:25 + mo])
            t_w2 = []
            for p4 in range(4):
                t = pw.tile([P, 4, 512], f32r, tag="w")
                nc.sync.dma_start(
                    t[:],
                    dw["ffn_w2"][l, p4 * 512:(p4 + 1) * 512, :]
                    .rearrange("(k p) n -> p k n", p=P))
                t_w2.append(t)
            t_y = pa.tile([P, 4, QH], f32r, tag="y")
            for mo in range(4):
                ps = psB.tile([P, 512], f32, tag="mm")
                for p4 in range(4):
                    for ki in range(4):
                        nc.tensor.matmul(ps[:, 0:QH],
                                         r(t_w2[p4][:, ki, mo * P:(mo + 1) * P]),
                                         r(t_h[:, p4 * 4 + ki, :]),
                                         start=(p4 == 0 and ki == 0),
                                         stop=(p4 == 3 and ki == 3))
                nc.vector.scalar_tensor_tensor(
                    out=t_y[:, mo, :], in0=ps[:, 0:QH],
                    scalar=t_sm[:, 40 + mo:41 + mo],
                    in1=resid[:, mo, :], op0=OP.add, op1=OP.add)
            resid = layernorm(t_y, t_sm, 60, 64)

            if l < L - 1:
                nc.sync.dma_start(
                    cc_in[l].ap().rearrange("(k p) q -> p k q", p=P), resid[:])
                nc.gpsimd.collective_compute(
                    "AllGather", OP.bypass, replica_groups=PAIRS,
                    ins=[cc_in[l].ap().opt()], outs=[cc_out[l].ap().opt()])

        # ================= epilogue =================
        # out_half: transpose resid -> token-major
        t_out = pa.tile([P, 2, D], f32, tag="outT", bufs=1)
        for ko in range(4):
            for qc in range(2):
                pst = psB.tile([P, 512], f32, tag="mm")
                nc.tensor.transpose(pst[:, 0:P].bitcast(f32r),
                                    resid[:, ko, qc * P:(qc + 1) * P],
                                    t_ident[:])
                nc.vector.tensor_copy(t_out[:, qc, ko * P:(ko + 1) * P],
                                      pst[:, 0:P])
        nc.sync.dma_start(d_out.ap().rearrange("(qc p) d -> p qc d", p=P),
                          t_out[:])
        # p_gens
        t_x0 = pa.tile([P, 4, QH], f32r, tag="x0T", bufs=1)
        nc.sync.dma_start(t_x0[:], d_x0T.ap().rearrange("(k p) q -> p k q", p=P))
        sr = psB.tile([P, 512], f32, tag="mm")
        chains = [(t_x0, 0), (resid, 4), (t_ctxT, 8)]
        n = 0
        for src, c0 in chains:
            for ko in range(4):
                nc.tensor.matmul(sr[0:1, 0:QH], r(t_pgw[:, c0 + ko:c0 + ko + 1]),
                                 r(src[:, ko, :]), start=(n == 0), stop=(n == 11))
                n += 1
        t_p = pa.tile([1, QH], f32, tag="pg", bufs=1)
        nc.scalar.activation(t_p[:], sr[0:1, 0:QH], AF.Sigmoid,
                             bias=t_pgw[0:1, 13:14].bitcast(f32),
                             scale=t_pgw[0:1, 12:13].bitcast(f32))
        nc.sync.dma_start(d_p.ap().rearrange("t o -> o t"), t_p[:])

    nc.compile()
    return nc


def _prep(inputs):
    """Host-side sharding + layout prep. Returns in_maps (one dict per core)."""
    gi = {k: np.ascontiguousarray(np.asarray(v, dtype=np.float32))
          for k, v in inputs.items()}
    x, enc = gi["x"], gi["enc_output"]
    maskT = np.ascontiguousarray(gi["look_ahead_mask"].T)

    sm = np.zeros((L, P, 72), np.float32)
    for l in range(L):
        def col(vec):
            return vec.reshape(-1, P).T
        sm[l, :, 0:4] = col(gi["sa_bq"][l])
        sm[l, :, 4:8] = col(gi["sa_bk"][l])
        sm[l, :, 8:12] = col(gi["sa_wo"][l].T @ gi["sa_bv"][l] + gi["sa_bo"][l])
        sm[l, :, 12:16] = col(gi["ca_bq"][l])
        sm[l, :, 16:20] = col(gi["ca_bk"][l])
        sm[l, :, 20:24] = col(gi["ca_wo"][l].T @ gi["ca_bv"][l] + gi["ca_bo"][l])
        sm[l, :, 24:40] = col(gi["ffn_b1"][l])
        sm[l, :, 40:44] = col(gi["ffn_b2"][l])
        for j in range(3):
            sm[l, :, 44 + 8 * j:48 + 8 * j] = col(gi["ln_g"][l, j])
            sm[l, :, 48 + 8 * j:52 + 8 * j] = col(gi["ln_b"][l, j])

    pgw = np.zeros((P, 16), np.float32)
    pgw[:, 0:4] = gi["wx_w"][:, 0].reshape(4, P).T
    pgw[:, 4:8] = gi["ws_w"][:, 0].reshape(4, P).T
    pgw[:, 8:12] = gi["wh_w"][:, 0].reshape(4, P).T
    v_w = float(gi["v_w"][0, 0])
    pgw[0, 12] = v_w
    pgw[0, 13] = (float(gi["wx_b"][0]) + float(gi["ws_b"][0])
                  + float(gi["wh_b"][0])) * v_w + float(gi["v_b"][0])

    weights = {k: gi[k] for k in ["sa_wq", "sa_wk", "sa_wv", "sa_wo",
                                  "ca_wq", "ca_wk", "ca_wv", "ca_wo",
                                  "ffn_w1", "ffn_w2"]}

    pm_all_zero = not np.any(gi["padding_mask"])

    in_maps = []
    for c in range(NCORES):
        b, hf = c // 2, c % 2
        xT = np.ascontiguousarray(x[b].T)
        encT = np.ascontiguousarray(enc[b].T)
        enc_pg = np.ascontiguousarray(
            enc[b].reshape(8, P, H, DH).transpose(2, 0, 1, 3))
        m = {
            "xT_in": xT,
            "x0T_half": np.ascontiguousarray(xT[:, hf * QH:(hf + 1) * QH]),
            "encT_in": encT,
            "enc_pg": enc_pg,
            "maskT_in": np.ascontiguousarray(maskT[:, hf * QH:(hf + 1) * QH]),
            "pm_bias": np.ascontiguousarray(
                gi["padding_mask"][b, 0, 0].reshape(8, P).T * -30000.0),
            "smalls": sm,
            "pg_w": pgw,
        }
        m.update(weights)
        in_maps.append(m)
    return in_maps, pm_all_zero


def _get_nc(pm_zero):
    key = ("nc", pm_zero)
    if key not in _CACHE:
        _CACHE[key] = _build(pm_zero)
    return _CACHE[key]


def kernel(**inputs):
    from concourse.bass_utils import run_bass_kernel_spmd

    in_maps, pm_zero = _prep(inputs)
    nc = _get_nc(pm_zero)
    res = run_bass_kernel_spmd(nc, in_maps, core_ids=list(range(NCORES)))
    outs, ps = [], []
    for b in range(B):
        outs.append(np.concatenate(
            [res.results[2 * b]["out_half"], res.results[2 * b + 1]["out_half"]],
            axis=0))
        ps.append(np.concatenate(
            [res.results[2 * b]["p_half"], res.results[2 * b + 1]["p_half"]],
            axis=0))
    return np.stack(outs), np.stack(ps)


# revision 20
# speedup vs baseline: 1.0710x; 1.0710x over previous
"""Trainium2 Bass kernel for nn_Decoder (4-layer transformer decoder with
pointer-generator head).

Sharding: 8 cores = 4 batches (data parallel) x 2 sequence halves.
Per layer a pair-AllGather shares the residual-stream halves so both cores
can compute full-sequence K/V for self-attention. Cross-attention K/V is
computed from enc_output locally (replicated within the pair, no comms).

Device layout: activations are kept feature-major (x^T: [D, tokens]) so every
matmul contracts over the partition dimension with no on-device transposes.
Attention computes S^T = K Q^T; softmax denominators come from a ones-column
appended to V (row 64 of the AV psum); per-token broadcasts are materialized
with K=1 matmuls. All matmuls run as float32r (fp32 storage, ~FP22 multiply).
"""

import numpy as np

L, D, H, F = 4, 512, 8, 2048
B, SQ, SK = 4, 512, 1024
DH = D // H
P = 128
NCORES = 8
QH = SQ // 2          # 256 tokens per core
EPS = 1e-6
MASK_COEF = -240000.0  # (mask*-240000 + S) then exp(x/8) == exp(S/8 - 30000*mask)
PAIRS = [[0, 1], [2, 3], [4, 5], [6, 7]]

_CACHE = {}


def _build(pm_zero: bool, repeat: int = 1, no_comm: bool = False, reuse_w: bool = False):
    import concourse.bass as bass
    import concourse.mybir as mybir
    import concourse.tile as tile
    from concourse import bacc
    from concourse.masks import make_identity

    f32 = mybir.dt.float32
    f32r = mybir.dt.float32r
    AF = mybir.ActivationFunctionType
    OP = mybir.AluOpType

    def r(ap):
        return ap.bitcast(f32r)

    nc = bacc.Bacc("TRN2", target_bir_lowering=False, debug=False,
                   num_devices=NCORES)

    # ---- DRAM I/O ----
    d_xT = nc.dram_tensor("xT_in", [D, SQ], f32r, kind="ExternalInput")
    d_x0T = nc.dram_tensor("x0T_half", [D, QH], f32r, kind="ExternalInput")
    d_encT = nc.dram_tensor("encT_in", [D, SK], f32r, kind="ExternalInput")
    d_encpg = nc.dram_tensor("enc_pg", [H, 8, P, DH], f32r, kind="ExternalInput")
    d_maskT = nc.dram_tensor("maskT_in", [SQ, QH], f32, kind="ExternalInput")
    d_pm = nc.dram_tensor("pm_bias", [P, 8], f32, kind="ExternalInput")
    d_sm = nc.dram_tensor("smalls", [L, P, 72], f32, kind="ExternalInput")
    d_pgw = nc.dram_tensor("pg_w", [P, 16], f32r, kind="ExternalInput")
    dw = {}
    for nm in ["sa_wq", "sa_wk", "sa_wv", "sa_wo",
               "ca_wq", "ca_wk", "ca_wv", "ca_wo"]:
        dw[nm] = nc.dram_tensor(nm, [L, D, D], f32r, kind="ExternalInput")
    dw["ffn_w1"] = nc.dram_tensor("ffn_w1", [L, D, F], f32r, kind="ExternalInput")
    dw["ffn_w2"] = nc.dram_tensor("ffn_w2", [L, F, D], f32r, kind="ExternalInput")

    d_out = nc.dram_tensor("out_half", [QH, D], f32, kind="ExternalOutput")
    d_p = nc.dram_tensor("p_half", [QH, 1], f32, kind="ExternalOutput")

    cc_in = [nc.dram_tensor(f"cc_in{l}", [D, QH], f32r) for l in range(L - 1)]
    cc_out = [nc.dram_tensor(f"cc_out{l}", [2 * D, QH], f32r) for l in range(L - 1)]

    from contextlib import ExitStack
    with tile.TileContext(nc) as tc, ExitStack() as ctx, \
            nc.allow_low_precision(reason="float32r tiles hold fp32 bits"):
        pw = ctx.enter_context(tc.tile_pool(name="w", bufs=4))
        pa = ctx.enter_context(tc.tile_pool(name="act", bufs=2))
        pp = ctx.enter_context(tc.tile_pool(name="pers", bufs=1))
        psA = ctx.enter_context(tc.tile_pool(name="psA", bufs=2, space="PSUM"))
        psB = ctx.enter_context(tc.tile_pool(name="psB", bufs=4, space="PSUM"))

        # ---- persistent tiles ----
        t_mask = pp.tile([P, 4, QH], f32)
        nc.sync.dma_start(t_mask[:], d_maskT.ap().rearrange("(k p) q -> p k q", p=P))
        t_encT = pp.tile([P, 4, SK], f32r)
        nc.sync.dma_start(t_encT[:], d_encT.ap().rearrange("(k p) t -> p k t", p=P))
        t_scr = pp.tile([P, P], f32)
        t_ones = pp.tile([P, P], f32r)
        nc.gpsimd.memset(t_scr[:], 1.0)
        nc.vector.tensor_copy(t_ones[:], t_scr[:])
        t_scr2 = pp.tile([P, P], f32)
        t_ident = pp.tile([P, P], f32r)
        make_identity(nc, t_scr2[:])
        nc.vector.tensor_copy(t_ident[:], t_scr2[:])
        t_pm = pp.tile([P, 8], f32)
        nc.sync.dma_start(t_pm[:], d_pm.ap())
        t_pgw = pp.tile([P, 16], f32r)
        nc.sync.dma_start(t_pgw[:], d_pgw.ap())
        t_ctxT = pp.tile([P, 4, QH], f32r)

        # initial residual = own half of x^T
        resid = pa.tile([P, 4, QH], f32r, tag="resid")
        nc.sync.dma_start(resid[:], d_x0T.ap().rearrange("(k p) q -> p k q", p=P))

        def load_w(dram_ap, key=None):
            t = pw.tile([P, 4, 512], f32r, tag="w")
            src = dram_ap.rearrange("(k p) n -> p k n", p=P)
            if reuse_w:
                nc.sync.dma_start(t[:, 0:1, :], src[:, 0:1, :])
            else:
                nc.sync.dma_start(t[:], src)
            return t

        def layernorm(t_y, t_sm, gcol, bcol):
            """t_y: [P,4,QH] pre-norm; returns new resid tile."""
            t_y2 = pa.tile([P, 4, QH], f32r, tag="y2", bufs=1)
            nc.scalar.square(t_y2[:].rearrange("p a b -> p (a b)"),
                             t_y[:].rearrange("p a b -> p (a b)"))
            row = psB.tile([P, 512], f32, tag="mm")
            for ko in range(4):
                nc.tensor.matmul(row[0:1, 0:QH], r(t_ones[:, 0:1]),
                                 r(t_y[:, ko, :]), start=(ko == 0), stop=(ko == 3))
            for ko in range(4):
                nc.tensor.matmul(row[0:1, QH:2 * QH], r(t_ones[:, 0:1]),
                                 r(t_y2[:, ko, :]), start=(ko == 0), stop=(ko == 3))
            t_r = pa.tile([1, 2, QH], f32r, tag="lnr")
            # m = ssum/D
            nc.vector.tensor_scalar_mul(t_r[0:1, 0, :], row[0:1, 0:QH], 1.0 / D)
            # var = ssq/D - m^2
            nc.vector.tensor_tensor(out=t_r[0:1, 1, :], in0=t_r[0:1, 0, :],
                                    in1=t_r[0:1, 0, :], op=OP.mult)
            nc.vector.scalar_tensor_tensor(out=t_r[0:1, 1, :],
                                           in0=row[0:1, QH:2 * QH],
                                           scalar=1.0 / D, in1=t_r[0:1, 1, :],
                                           op0=OP.mult, op1=OP.subtract)
            # rs = 1/sqrt(var+eps)
            nc.vector.tensor_scalar_add(t_r[0:1, 1, :], t_r[0:1, 1, :], EPS)
            nc.scalar.activation(t_r[0:1, 1, :], t_r[0:1, 1, :], AF.Sqrt)
            nc.vector.reciprocal(t_r[0:1, 1, :], t_r[0:1, 1, :])
            # broadcast [m | rs] -> [128, 2*QH]
            bc = psB.tile([P, 512], f32, tag="mm")
            nc.tensor.matmul(bc[:, 0:2 * QH], r(t_ones[0:1, 0:P]),
                             r(t_r[0:1, :, :].rearrange("p a b -> p (a b)")),
                             start=True, stop=True)
            newr = pa.tile([P, 4, QH], f32r, tag="resid")
            t1 = pa.tile([P, 4, QH], f32, tag="lnt", bufs=1)
            for mo in range(4):
                nc.vector.tensor_tensor(out=t1[:, mo, :], in0=t_y[:, mo, :],
                                        in1=bc[:, 0:QH], op=OP.subtract)
                nc.vector.tensor_tensor(out=t1[:, mo, :], in0=t1[:, mo, :],
                                        in1=bc[:, QH:2 * QH], op=OP.mult)
                nc.vector.tensor_scalar(out=newr[:, mo, :], in0=t1[:, mo, :],
                                        scalar1=t_sm[:, gcol + mo:gcol + mo + 1],
                                        scalar2=t_sm[:, bcol + mo:bcol + mo + 1],
                                        op0=OP.mult, op1=OP.add)
            return newr

        def make_qT(t_wq, t_sm, bq_c, resid):
            t_qT = pa.tile([P, 4, QH], f32r, tag="qT")
            for mo in range(4):
                ps = psB.tile([P, 512], f32, tag="mm")
                for ko in range(4):
                    nc.tensor.matmul(ps[:, 0:QH],
                                     r(t_wq[:, ko, mo * P:(mo + 1) * P]),
                                     r(resid[:, ko, :]),
                                     start=(ko == 0), stop=(ko == 3))
                nc.scalar.activation(t_qT[:, mo, :], ps[:, 0:QH], AF.Identity,
                                     bias=t_sm[:, bq_c + mo:bq_c + mo + 1])
            return t_qT

        def make_kv(t_wk, t_wv, t_sm, bk_c, kv_src, nt, sfx):
            Tk = nt * P
            t_kT = pa.tile([P, 4, Tk], f32r, tag="kT" + sfx, bufs=1)
            for mo in range(4):
                for hh in range(Tk // 512):
                    ps = psB.tile([P, 512], f32, tag="mm")
                    for ko in range(4):
                        nc.tensor.matmul(ps[:, :],
                                         r(t_wk[:, ko, mo * P:(mo + 1) * P]),
                                         r(kv_src[:, ko, hh * 512:(hh + 1) * 512]),
                                         start=(ko == 0), stop=(ko == 3))
                    nc.scalar.activation(t_kT[:, mo, hh * 512:(hh + 1) * 512],
                                         ps[:, :], AF.Identity,
                                         bias=t_sm[:, bk_c + mo:bk_c + mo + 1])
            t_vp = pa.tile([P, nt, H, DH + 1], f32r, tag="vp" + sfx, bufs=1)
            nc.vector.tensor_scalar(
                out=t_vp[:, 0:nt, :, DH:DH + 1],
                in0=t_ones[:, 0:nt * H].rearrange("p (a b c) -> p a b c",
                                                  b=H, c=1),
                scalar1=0.0, scalar2=1.0, op0=OP.mult, op1=OP.add)
            for mo in range(nt):
                ps = psB.tile([P, 512], f32, tag="mm")
                for ko in range(4):
                    nc.tensor.matmul(ps[:, :],
                                     r(kv_src[:, ko, mo * P:(mo + 1) * P]),
                                     r(t_wv[:, ko, :]),
                                     start=(ko == 0), stop=(ko == 3))
                nc.vector.tensor_copy(t_vp[:, mo, :, 0:DH],
                                      ps[:, :].rearrange("p (h d) -> p h d", d=DH))
            return t_kT, t_vp

        def attention(t_qT, t_kT, t_vp, t_wo, t_sm, co_c,
                      nt, is_sa, save_ctx, resid):
            """MHA core; returns t_y (pre-LN = attn_out + c_o + resid)."""
            t_O = pa.tile([P, 4, QH], f32r, tag="Ofull", bufs=1)
            for h in range(H):
                base = (h % 2) * DH
                moh = h // 2
                es = pa.tile([P, 8, QH], f32r, tag="es")
                for g in range(nt // 4):
                    sp = psA.tile([P, 1024], f32, tag="sc")
                    for kc in range(4):
                        kcg = g * 4 + kc
                        nc.tensor.matmul(
                            sp[:, kc * QH:(kc + 1) * QH],
                            r(t_kT[base:base + DH, moh, kcg * P:(kcg + 1) * P]),
                            r(t_qT[base:base + DH, moh, :]),
                            start=True, stop=True)
                    spv = sp[:, :].rearrange("p (k q) -> p k q", q=QH)
                    if is_sa:
                        nc.vector.scalar_tensor_tensor(
                            out=es[:, g * 4:(g + 1) * 4, :], in0=t_mask[:, :, :],
                            scalar=MASK_COEF, in1=spv,
                            op0=OP.mult, op1=OP.add)
                        nc.scalar.activation(es[:, g * 4:(g + 1) * 4, :],
                                             es[:, g * 4:(g + 1) * 4, :],
                                             AF.Exp, scale=0.125)
                    elif pm_zero:
                        nc.scalar.activation(es[:, g * 4:(g + 1) * 4, :], spv,
                                             AF.Exp, scale=0.125)
                    else:
                        for kc in range(4):
                            kcg = g * 4 + kc
                            nc.scalar.activation(
                                es[:, kcg, :], sp[:, kc * QH:(kc + 1) * QH],
                                AF.Exp, scale=0.125,
                                bias=t_pm[:, kcg:kcg + 1])
                op = psB.tile([P, 512], f32, tag="mm")
                for kc in range(nt):
                    nc.tensor.matmul(op[0:DH + 1, 0:QH], r(t_vp[:, kc, h, :]),
                                     r(es[:, kc, :]),
                                     start=(kc == 0), stop=(kc == nt - 1))
                t_rd = pa.tile([P, QH], f32r, tag="rd", bufs=2)
                nc.vector.reciprocal(t_rd[64:65, :], op[64:65, 0:QH])
                bc = psB.tile([P, 512], f32, tag="mm")
                nc.tensor.matmul(bc[0:DH, 0:QH], r(t_ones[64:65, 0:DH]),
                                 r(t_rd[64:65, :]), start=True, stop=True)
                rb = pa.tile([DH, QH], f32, tag="rb")
                nc.vector.tensor_copy(rb[:], bc[0:DH, 0:QH])
                nc.vector.tensor_tensor(out=t_O[base:base + DH, moh, :],
                                        in0=op[0:DH, 0:QH], in1=rb[:],
                                        op=OP.mult)
                if save_ctx:
                    t_ep = pa.tile([P, 8, DH], f32r, tag="ep")
                    nc.sync.dma_start(t_ep[:],
                                      d_encpg[h].rearrange("k p d -> p k d"))
                    cp = psB.tile([P, 512], f32, tag="mm")
                    for kc in range(8):
                        nc.tensor.matmul(cp[0:DH, 0:QH], r(t_ep[:, kc, :]),
                                         r(es[:, kc, :]),
                                         start=(kc == 0), stop=(kc == 7))
                    nc.vector.tensor_tensor(out=t_ctxT[base:base + DH, moh, :],
                                            in0=cp[0:DH, 0:QH],
                                            in1=rb[:], op=OP.mult)
            # wo + folded bias + residual
            t_y = pa.tile([P, 4, QH], f32r, tag="y")
            for mo in range(4):
                ps = psB.tile([P, 512], f32, tag="mm")
                for ko in range(4):
                    nc.tensor.matmul(ps[:, 0:QH],
                                     r(t_wo[:, ko, mo * P:(mo + 1) * P]),
                                     r(t_O[:, ko, :]),
                                     start=(ko == 0), stop=(ko == 3))
                nc.vector.scalar_tensor_tensor(
                    out=t_y[:, mo, :], in0=ps[:, 0:QH],
                    scalar=t_sm[:, co_c + mo:co_c + mo + 1],
                    in1=resid[:, mo, :], op0=OP.add, op1=OP.add)
            return t_y

        # ================= layer loop =================
        for rep in range(repeat):
          for l in range(L):
                t_xT = pa.tile([P, 4, SQ], f32r, tag="xTfull", bufs=1)
                if l == 0:
                    nc.sync.dma_start(t_xT[:],
                                      d_xT.ap().rearrange("(k p) t -> p k t", p=P))
                else:
                    nc.sync.dma_start(
                        t_xT[:, :, 0:QH],
                        cc_out[l - 1][0:D, :].rearrange("(k p) q -> p k q", p=P))
                    nc.sync.dma_start(
                        t_xT[:, :, QH:SQ],
                        cc_out[l - 1][D:2 * D, :].rearrange("(k p) q -> p k q", p=P))
                t_sm = pa.tile([P, 72], f32, tag="sm")
                nc.sync.dma_start(t_sm[:], d_sm[l])

                # xT-independent first: ca k/v, sa q (covers the AllGather)
                t_wk_ca = load_w(dw["ca_wk"][l], key=("k", False))
                t_wv_ca = load_w(dw["ca_wv"][l], key=("v", False))
                kT_ca, vp_ca = make_kv(t_wk_ca, t_wv_ca, t_sm, 16, t_encT, 8, "c")
                t_wq_sa = load_w(dw["sa_wq"][l], key=("q", True))
                qT_sa = make_qT(t_wq_sa, t_sm, 0, resid)
                # sa k/v (needs gathered xT)
                t_wk_sa = load_w(dw["sa_wk"][l], key=("k", True))
                t_wv_sa = load_w(dw["sa_wv"][l], key=("v", True))
                kT_sa, vp_sa = make_kv(t_wk_sa, t_wv_sa, t_sm, 4, t_xT, 4, "s")
                t_wo_sa = load_w(dw["sa_wo"][l], key=("o", True))
                t_y = attention(qT_sa, kT_sa, vp_sa, t_wo_sa, t_sm, 8,
                                4, True, False, resid)
                resid = layernorm(t_y, t_sm, 44, 48)
                # cross attention
                t_wq_ca = load_w(dw["ca_wq"][l], key=("q", False))
                qT_ca = make_qT(t_wq_ca, t_sm, 12, resid)
                t_wo_ca = load_w(dw["ca_wo"][l], key=("o", False))
                t_y = attention(qT_ca, kT_ca, vp_ca, t_wo_ca, t_sm, 20,
                                8, False, (l == L - 1), resid)
                resid = layernorm(t_y, t_sm, 52, 56)
                # ffn
                t_h = pa.tile([P, 16, QH], f32r, tag="hT", bufs=1)
                for p4 in range(4):
                    t_w1 = pw.tile([P, 4, 512], f32r, tag="w")
                    nc.sync.dma_start(
                        t_w1[:],
                        dw["ffn_w1"][l, :, p4 * 512:(p4 + 1) * 512]
                        .rearrange("(k p) n -> p k n", p=P))
                    for mi in range(4):
                        mo = p4 * 4 + mi
                        ps = psB.tile([P, 512], f32, tag="mm")
                        for ko in range(4):
                            nc.tensor.matmul(ps[:, 0:QH],
                                             r(t_w1[:, ko, mi * P:(mi + 1) * P]),
                                             r(resid[:, ko, :]),
                                             start=(ko == 0), stop=(ko == 3))
                        nc.scalar.activation(t_h[:, mo, :], ps[:, 0:QH], AF.Relu,
                                             bias=t_sm[:, 24 + mo:25 + mo])
                t_w2 = []
                for p4 in range(4):
                    t = pw.tile([P, 4, 512], f32r, tag="w")
                    nc.sync.dma_start(
                        t[:],
                        dw["ffn_w2"][l, p4 * 512:(p4 + 1) * 512, :]
                        .rearrange("(k p) n -> p k n", p=P))
                    t_w2.append(t)
                t_y = pa.tile([P, 4, QH], f32r, tag="y")
                for mo in range(4):
                    ps = psB.tile([P, 512], f32, tag="mm")
                    for p4 in range(4):
                        for ki in range(4):
                            nc.tensor.matmul(ps[:, 0:QH],
                                             r(t_w2[p4][:, ki, mo * P:(mo + 1) * P]),
                                             r(t_h[:, p4 * 4 + ki, :]),
                                             start=(p4 == 0 and ki == 0),
                                             stop=(p4 == 3 and ki == 3))
                    nc.vector.scalar_tensor_tensor(
                        out=t_y[:, mo, :], in0=ps[:, 0:QH],
                        scalar=t_sm[:, 40 + mo:41 + mo],
                        in1=resid[:, mo, :], op0=OP.add, op1=OP.add)
                resid = layernorm(t_y, t_sm, 60, 64)

                if l < L - 1:
                    nc.sync.dma_start(
                        cc_in[l].ap().rearrange("(k p) q -> p k q", p=P), resid[:])
                    nc.gpsimd.collective_compute(
                        "AllGather", OP.bypass, replica_groups=PAIRS,
                        ins=[cc_in[l].ap().opt()], outs=[cc_out[l].ap().opt()])

        # ================= epilogue =================
        # out_half: transpose resid -> token-major
        t_out = pa.tile([P, 2, D], f32, tag="outT", bufs=1)
        for ko in range(4):
            for qc in range(2):
                pst = psB.tile([P, 512], f32, tag="mm")
                nc.tensor.transpose(pst[:, 0:P].bitcast(f32r),
                                    resid[:, ko, qc * P:(qc + 1) * P],
                                    t_ident[:])
                nc.vector.tensor_copy(t_out[:, qc, ko * P:(ko + 1) * P],
                                      pst[:, 0:P])
        nc.sync.dma_start(d_out.ap().rearrange("(qc p) d -> p qc d", p=P),
                          t_out[:])
        # p_gens
        t_x0 = pa.tile([P, 4, QH], f32r, tag="x0T", bufs=1)
        nc.sync.dma_start(t_x0[:], d_x0T.ap().rearrange("(k p) q -> p k q", p=P))
        sr = psB.tile([P, 512], f32, tag="mm")
        chains = [(t_x0, 0), (resid, 4), (t_ctxT, 8)]
        n = 0
        for src, c0 in chains:
            for ko in range(4):
                nc.tensor.matmul(sr[0:1, 0:QH], r(t_pgw[:, c0 + ko:c0 + ko + 1]),
                                 r(src[:, ko, :]), start=(n == 0), stop=(n == 11))
                n += 1
        t_p = pa.tile([1, QH], f32, tag="pg", bufs=1)
        nc.scalar.activation(t_p[:], sr[0:1, 0:QH], AF.Sigmoid,
                             bias=t_pgw[0:1, 13:14].bitcast(f32),
                             scale=t_pgw[0:1, 12:13].bitcast(f32))
        nc.sync.dma_start(d_p.ap().rearrange("t o -> o t"), t_p[:])

    nc.compile()
    return nc


def _prep(inputs):
    """Host-side sharding + layout prep. Returns in_maps (one dict per core)."""
    gi = {k: np.ascontiguousarray(np.asarray(v, dtype=np.float32))
          for k, v in inputs.items()}
    x, enc = gi["x"], gi["enc_output"]
    maskT = np.ascontiguousarray(gi["look_ahead_mask"].T)

    sm = np.zeros((L, P, 72), np.float32)
    for l in range(L):
        def col(vec):
            return vec.reshape(-1, P).T
        sm[l, :, 0:4] = col(gi["sa_bq"][l])
        sm[l, :, 4:8] = col(gi["sa_bk"][l])
        sm[l, :, 8:12] = col(gi["sa_wo"][l].T @ gi["sa_bv"][l] + gi["sa_bo"][l])
        sm[l, :, 12:16] = col(gi["ca_bq"][l])
        sm[l, :, 16:20] = col(gi["ca_bk"][l])
        sm[l, :, 20:24] = col(gi["ca_wo"][l].T @ gi["ca_bv"][l] + gi["ca_bo"][l])
        sm[l, :, 24:40] = col(gi["ffn_b1"][l])
        sm[l, :, 40:44] = col(gi["ffn_b2"][l])
        for j in range(3):
            sm[l, :, 44 + 8 * j:48 + 8 * j] = col(gi["ln_g"][l, j])
            sm[l, :, 48 + 8 * j:52 + 8 * j] = col(gi["ln_b"][l, j])

    pgw = np.zeros((P, 16), np.float32)
    pgw[:, 0:4] = gi["wx_w"][:, 0].reshape(4, P).T
    pgw[:, 4:8] = gi["ws_w"][:, 0].reshape(4, P).T
    pgw[:, 8:12] = gi["wh_w"][:, 0].reshape(4, P).T
    v_w = float(gi["v_w"][0, 0])
    pgw[0, 12] = v_w
    pgw[0, 13] = (float(gi["wx_b"][0]) + float(gi["ws_b"][0])
                  + float(gi["wh_b"][0])) * v_w + float(gi["v_b"][0])

    weights = {k: gi[k] for k in ["sa_wq", "sa_wk", "sa_wv", "sa_wo",
                                  "ca_wq", "ca_wk", "ca_wv", "ca_wo",
                                  "ffn_w1", "ffn_w2"]}

    pm_all_zero = not np.any(gi["padding_mask"])

    in_maps = []
    for c in range(NCORES):
        b, hf = c // 2, c % 2
        xT = np.ascontiguousarray(x[b].T)
        encT = np.ascontiguousarray(enc[b].T)
        enc_pg = np.ascontiguousarray(
            enc[b].reshape(8, P, H, DH).transpose(2, 0, 1, 3))
        m = {
            "xT_in": xT,
            "x0T_half": np.ascontiguousarray(xT[:, hf * QH:(hf + 1) * QH]),
            "encT_in": encT,
            "enc_pg": enc_pg,
            "maskT_in": np.ascontiguousarray(maskT[:, hf * QH:(hf + 1) * QH]),
            "pm_bias": np.ascontiguousarray(
                gi["padding_mask"][b, 0, 0].reshape(8, P).T * -30000.0),
            "smalls": sm,
            "pg_w": pgw,
        }
        m.update(weights)
        in_maps.append(m)
    return in_maps, pm_all_zero


def _get_nc(pm_zero):
    key = ("nc", pm_zero)
    if key not in _CACHE:
        _CACHE[key] = _build(pm_zero)
    return _CACHE[key]


def kernel(**inputs):
    from concourse.bass_utils import run_bass_kernel_spmd

    in_maps, pm_zero = _prep(inputs)
    nc = _get_nc(pm_zero)
    res = run_bass_kernel_spmd(nc, in_maps, core_ids=list(range(NCORES)))
    outs, ps = [], []
    for b in range(B):
        outs.append(np.concatenate(
            [res.results[2 * b]["out_half"], res.results[2 * b + 1]["out_half"]],
            axis=0))
        ps.append(np.concatenate(
            [res.results[2 * b]["p_half"], res.results[2 * b + 1]["p_half"]],
            axis=0))
    return np.stack(outs), np.stack(ps)
